# revision 36
# baseline (speedup 1.0000x reference)
"""Causal multi-head attention (B=8, L=1024, D_IN=512, H=8, D=64) on 8 TRN2
NeuronCores, data-parallel over batch (one batch element per core, no
collectives).

Every matmul runs in the SAME 64x128 row-tiled PE mode (no tiling-mode
switch drains), with the two row tiles T0 (SBUF partitions 0:64) and T8
(64:128) streaming CONCURRENTLY into different PSUM banks:

  proj:   qT/kT/v chunks contract K=512 as 4 K=64 subtiles per row tile;
          T0 accumulates bank A, T8 bank B, DVE adds A+B -> SBUF bf16.
  scores: head pair t lives on partition halves of qT/kT, so T0 computes
          head 2t and T8 head 2t+1 in parallel -> PSUM [128, 2, 512].
  exp:    ONE ScalarE activation over both heads' banks -> pexp SBUF bf16;
          causal diagonal 128x128 block masked by a DVE 0/1 multiply.
  AV:     cross passes: (T0: v_h keys-lo -> oT_h) || (T8: v_h' keys-hi ->
          oT_h'), then swapped, accumulating per-head oT [66, 512] banks
          over key tiles (ones columns in v carry the softmax denominator).

The attention loop is query-windowed (qc of 512 cols) and software-
pipelined: AV lags scores by TWO key tiles so the scalar-engine exp and
the DVE mask never block the PE's FIFO. Projection chunks for the next
head pair ride in the PE's idle slots (ScalarE is the saturated engine).

host: QsT/KsT/VsT = seq[b].T bf16; WQ pre-scaled by 1/sqrt(D);
      OUT[h, :64, :] / OUT[h, 64, :], transpose, concat heads.
"""

import numpy as np
import ml_dtypes

B, L, D_IN = 8, 1024, 512
H, D = 8, 64
DA = D + 2  # head dim + two ones columns (denominator; padded even so the
# bf16 lhsT slices stay 4-byte aligned -- odd column counts hang the HW)
N_CORES = 8
SCALE = 1.0 / np.sqrt(D).item()  # folded into WQ on the host
N_WARMUP = 22  # dummy matmuls to open the HAM clock gate during input DMA
# (sized so the warmup chain ends roughly when the first projection's
# inputs land ~18us in: keeps the PE continuously busy so HAM stays open)

_GRAPH_CACHE = {}


def build_attention_body(tc, qsT, ksT, vsT, wq, wk, wv, mask2, out):
    """Emit the per-core kernel into TileContext `tc` (APs per module doc)."""
    import contextlib

    import concourse.mybir as mybir

    nc = tc.nc
    fp32 = mybir.dt.float32
    bf16 = mybir.dt.bfloat16
    fp8 = mybir.dt.float8e4
    DR = mybir.MatmulPerfMode.DoubleRow
    EXP = mybir.ActivationFunctionType.Exp

    with contextlib.ExitStack() as ctx:
        const = ctx.enter_context(tc.tile_pool(name="const", bufs=1))
        sb = ctx.enter_context(tc.tile_pool(name="sb", bufs=1))
        ppool = ctx.enter_context(tc.tile_pool(name="ppool", bufs=1))
        stage = ctx.enter_context(tc.tile_pool(name="stage", bufs=1))
        psum = ctx.enter_context(tc.tile_pool(name="psum", bufs=2, space="PSUM"))

        # ---- ScalarE exp-table preload + PE warm-up racing the input DMAs --
        # (warmup borrows the "S" psum tag so proj chunks never wait on it)
        warm_sb = const.tile([128, 512], bf16)
        nc.vector.memset(warm_sb[:], 0.0)
        warm_out = const.tile([128, 8], bf16)
        nc.scalar.activation(warm_out[:], warm_sb[:, 0:8], EXP)
        pwarm = psum.tile([128, 2, 512], fp32, tag="S", bufs=2, name="pwarm")
        for i in range(N_WARMUP):
            nc.tensor.matmul(
                pwarm[:, i % 2, :], warm_sb[:, 0:128], warm_sb[:],
                start=True, stop=True, skip_group_check=True,
            )

        # ---- stage inputs into SBUF. Host pre-rearranged every tensor to
        # [128, ...] partition-major, so each DMA moves fully contiguous
        # 2-8 KB lines per partition. The two HWDGE rings (sync + ScalarE)
        # carry the q/v- and k-paths in parallel, seq tensors split in
        # halves ordered by first use so the first projection starts early.
        wq_r = wq.rearrange("p (kt n) -> p kt n", kt=4)
        wk_r = wk.rearrange("p (kt n) -> p kt n", kt=4)
        qsT_r = qsT.rearrange("p (kt h l) -> p kt h l", kt=4, h=2)
        ksT_r = ksT.rearrange("p (kt h l) -> p kt h l", kt=4, h=2)
        vsT_r = vsT.rearrange("p (kt h l) -> p kt h l", kt=4, h=2)
        wq_sb = const.tile([128, 4, 512], bf16)
        qsT_sb = const.tile([128, 4, L], bf16)
        wk_sb = const.tile([128, 4, 512], bf16)
        ksT_sb = const.tile([128, 4, L], bf16)
        wv_sb = const.tile([128, 4, 512], bf16)
        vsT_sb = const.tile([128, 4, L], bf16)
        mask_sb = const.tile([128, 2, 128], bf16)
        # the pair-0 first-window slices jump the queue on each ring so the
        # first projection/scores start as early as HBM allows
        nc.sync.dma_start(wq_sb[:, :, 0:128], wq_r[:, :, 0:128])
        nc.scalar.dma_start(wk_sb[:, :, 0:128], wk_r[:, :, 0:128])
        nc.sync.dma_start(qsT_sb[:, :, 0:512], qsT_r[:, :, 0])
        nc.scalar.dma_start(ksT_sb[:, :, 0:512], ksT_r[:, :, 0])
        nc.sync.dma_start(wq_sb[:, :, 128:512], wq_r[:, :, 128:512])
        nc.scalar.dma_start(wk_sb[:, :, 128:512], wk_r[:, :, 128:512])
        nc.sync.dma_start(wv_sb[:], wv.rearrange("p (kt n) -> p kt n", kt=4))
        nc.scalar.dma_start(ksT_sb[:, :, 512:L], ksT_r[:, :, 1])
        nc.sync.dma_start(vsT_sb[:, :, 0:512], vsT_r[:, :, 0])
        nc.scalar.dma_start(mask_sb[:], mask2[:, :, :])
        nc.sync.dma_start(qsT_sb[:, :, 512:L], qsT_r[:, :, 1])
        nc.sync.dma_start(vsT_sb[:, :, 512:L], vsT_r[:, :, 1])

        # ---- persistent activations -------------------------------------
        qT_sb = sb.tile([128, 4, L], bf16)   # [dout%128, pair, L]
        # kT zero-padded per head: kTz[:, z, t, :] holds head 2t+z's 64 dims
        # on its own partition half and ZEROS on the other, so score matmuls
        # contract K=128 -- the same PE tiling mode as every other matmul
        # (no 64x128 <-> 128x128 mode-switch drains on the PE)
        kTz_sb = sb.tile([128, 2, 4, L], bf16)
        v_sb = sb.tile([128, 8, H, DA], bf16)  # [j%128, j//128, head, d|1|1]
        # ones everywhere; proj overwrites [:, :, :, 0:64], cols 64:66 stay 1
        nc.vector.memset(v_sb[:], 1.0)
        # per-partition selectors: 1 on the head's own partition half, 0 on
        # the other -- the kT copy then zero-pads kTz with NO big memset
        # gating the first scores (0 * finite = 0 exactly)
        halfsel = const.tile([128, 2], fp32)
        nc.vector.memset(halfsel[:], 0.0)
        nc.vector.memset(halfsel[0:64, 0:1], 1.0)
        nc.vector.memset(halfsel[64:128, 1:2], 1.0)

        def proj_qk_chunk(t, which, nch):
            # one [128, 512] chunk of qT (which=0) / kT (which=1), pair t
            # (projections contract K=128 full-array; DVE can't add two PSUM
            # banks, so the 64x128 split would double the streamed columns)
            w_t, src = ((wq_sb, qsT_sb), (wk_sb, ksT_sb))[which]
            pq = psum.tile(
                [128, 512], fp32, tag="work", bufs=2,
                name=f"pq_{t}_{which}_{nch}",
            )
            cols = slice(nch * 512, (nch + 1) * 512)
            for kt in range(4):
                nc.tensor.matmul(
                    pq[:],
                    w_t[:, kt, t * 128:(t + 1) * 128],
                    src[:, kt, cols],
                    start=(kt == 0),
                    stop=(kt == 3),
                )
            if which == 0:
                nc.vector.tensor_copy(out=qT_sb[:, t, cols], in_=pq[:])
            else:
                # each head's 64 dims land in its slot, the other half
                # zeroed by the selector
                for z in range(2):
                    nc.vector.tensor_scalar_mul(
                        out=kTz_sb[:, z, t, cols], in0=pq[:],
                        scalar1=halfsel[:, z:z + 1],
                    )

        def proj_v(it):
            # v natural: v[i, n] = sum_k Vs[i, k] WV[k, n]; lhsT = VsT tile
            pv = psum.tile([128, 512], fp32, tag="work", bufs=2,
                           name=f"pv_{it}")
            for kt in range(4):
                nc.tensor.matmul(
                    pv[:],
                    vsT_sb[:, kt, it * 128:(it + 1) * 128],
                    wv_sb[:, kt, :],
                    start=(kt == 0),
                    stop=(kt == 3),
                )
            nc.vector.tensor_copy(
                out=v_sb[:, it, :, 0:D],
                in_=pv.rearrange("p (h d) -> p h d", h=H),
            )

        # proj work interleaved into the attention jt loops so the PE has
        # useful work while ScalarE (the critical engine) drains exps
        fillers = []
        for which in range(2):
            fillers.append(lambda w=which: proj_qk_chunk(0, w, 1))
        for it in range(4, 8):
            fillers.append(lambda it=it: proj_v(it))
        for t in range(1, 4):
            for which in range(2):
                for nch in range(2):
                    fillers.append(
                        (lambda t=t, w=which, n=nch: proj_qk_chunk(t, w, n))
                    )

        # spread proj fillers across the 48 attention steps (front-loading
        # them makes the first half PE-bound and the back half ScalarE-
        # starved); each lands before the pair that consumes it starts
        filler_steps = {0, 2, 3, 5, 6, 7, 8, 9, 10, 11,
                        14, 17, 20, 23, 26, 29, 32, 35}
        step_counter = [0]

        def emit_filler():
            if fillers and step_counter[0] in filler_steps:
                fillers.pop(0)()
            step_counter[0] += 1

        def emit_av(t, jt, last_jt, oT, pexp, qoff, cw):
            for hh in range(2):
                nc.tensor.matmul(
                    oT[hh][:, qoff:qoff + cw],
                    v_sb[:, jt, 2 * t + hh, :],
                    pexp[:, hh, 0:cw],
                    start=(jt == 0),
                    stop=(jt == last_jt),
                    skip_group_check=True,
                )

        def attention_pair(t):
            # pair 3 does the big window first so the kernel tail is short
            for qc in ((1, 0) if t == 3 else (0, 1)):
                oT = [
                    psum.tile([DA, 512], fp32, tag="oT", bufs=2,
                              name=f"oT_{t}_{qc}_{hh}")
                    for hh in range(2)
                ]
                last_jt = 4 * qc + 3
                pipe = []
                for jt in range(last_jt + 1):
                    j0 = 128 * jt
                    lo = max(j0, 512 * qc)
                    cw = 512 * qc + 512 - lo
                    qoff = lo - 512 * qc
                    diag = (lo == j0)
                    ps = psum.tile(
                        [128, 2, 512], fp32, tag="S", bufs=2,
                        name=f"S_{t}_{qc}_{jt}",
                    )
                    for hh in range(2):
                        nc.tensor.matmul(
                            ps[:, hh, 0:cw],
                            kTz_sb[:, hh, t, j0:j0 + 128],
                            qT_sb[:, t, lo:lo + cw],
                            start=True,
                            stop=True,
                            skip_group_check=True,
                        )
                    pexp = ppool.tile(
                        [128, 2, 512], bf16, tag="P", bufs=6,
                        name=f"P_{t}_{qc}_{jt}",
                    )
                    nc.scalar.activation(pexp[:, :, 0:cw], ps[:, :, 0:cw], EXP)
                    if diag:
                        # causal 0/1 mask on the diagonal 128x128 block
                        nc.vector.tensor_mul(
                            pexp[:, :, 0:128], pexp[:, :, 0:128], mask_sb[:]
                        )
                    pipe.append((t, jt, last_jt, oT, pexp, qoff, cw))
                    emit_filler()
                    if len(pipe) > 2:  # AV lags scores by two key tiles
                        emit_av(*pipe.pop(0))
                while pipe:
                    emit_av(*pipe.pop(0))
                for hh in range(2):
                    o_st = stage.tile(
                        [DA, 512], bf16, tag="ost", bufs=4,
                        name=f"ost_{t}_{qc}_{hh}",
                    )
                    nc.vector.tensor_copy(out=o_st[:], in_=oT[hh][:])
                    nc.sync.dma_start(
                        out[2 * t + hh, :, 512 * qc:512 * qc + 512], o_st[:]
                    )

        # emit: pair-0 q/k first-window projections (scores start earliest),
        # then v tiles 0..3 (first AVs); the rest ride the filler queue
        for which in range(2):
            proj_qk_chunk(0, which, 0)
        for it in range(4):
            proj_v(it)
        for t in range(4):
            attention_pair(t)


def _build_graph():
    import concourse.mybir as mybir
    import concourse.tile as tile
    from concourse import bacc

    nc = bacc.Bacc("TRN2", target_bir_lowering=False)
    bf16 = mybir.dt.bfloat16
    # all inputs pre-rearranged on the host to partition-major [128, ...]
    qsT = nc.dram_tensor("QsT", (128, 4 * L), bf16, kind="ExternalInput")
    ksT = nc.dram_tensor("KsT", (128, 4 * L), bf16, kind="ExternalInput")
    vsT = nc.dram_tensor("VsT", (128, 4 * L), bf16, kind="ExternalInput")
    wq = nc.dram_tensor("WQ", (128, 4 * 512), bf16, kind="ExternalInput")
    wk = nc.dram_tensor("WK", (128, 4 * 512), bf16, kind="ExternalInput")
    wv = nc.dram_tensor("WV", (128, 4 * 512), bf16, kind="ExternalInput")
    mask2 = nc.dram_tensor("MASK2", (128, 2, 128), bf16, kind="ExternalInput")
    out = nc.dram_tensor("OUT", (H, DA, L), bf16, kind="ExternalOutput")

    with tile.TileContext(nc) as tc:
        build_attention_body(
            tc, qsT[:], ksT[:], vsT[:], wq[:], wk[:], wv[:], mask2[:], out[:],
        )
    nc.compile()
    return nc


def get_graph():
    if "nc" not in _GRAPH_CACHE:
        _GRAPH_CACHE["nc"] = _build_graph()
    return _GRAPH_CACHE["nc"]


def _prep_seq(x):
    """[L, 512] -> partition-major [128, 4*L] bf16 (x.T tiled by 128 rows)."""
    xT = np.asarray(x, np.float32).T.reshape(4, 128, L)
    return np.ascontiguousarray(
        xT.transpose(1, 0, 2).reshape(128, 4 * L)
    ).astype(ml_dtypes.bfloat16)


def _prep_w(w, scale=1.0):
    """[512, 512] -> partition-major [128, 4*512] bf16."""
    w32 = (np.asarray(w, np.float32) * scale).reshape(4, 128, 512)
    return np.ascontiguousarray(
        w32.transpose(1, 0, 2).reshape(128, 4 * 512)
    ).astype(ml_dtypes.bfloat16)


def make_in_maps(Q_seq, K_seq, V_seq, WQ, WK, WV):
    bf = ml_dtypes.bfloat16
    # fold the softmax 1/sqrt(D) into WQ so no scale is needed on-device
    # (fp8 projections were tried and give 4.5e-2 rel err -- over budget)
    wq = _prep_w(WQ, SCALE)
    wk = _prep_w(WK)
    wv = _prep_w(WV)
    # keep-mask in S^T block coords, duplicated per head of the pair:
    # keep key <= query  <=>  row r (key) <= col c (query)
    m = np.triu(np.ones((128, 128), np.float32))
    mask2 = np.ascontiguousarray(
        np.broadcast_to(m[:, None, :], (128, 2, 128))
    ).astype(bf)
    in_maps = []
    for b in range(N_CORES):
        in_maps.append({
            "QsT": _prep_seq(Q_seq[b]),
            "KsT": _prep_seq(K_seq[b]),
            "VsT": _prep_seq(V_seq[b]),
            "WQ": wq,
            "WK": wk,
            "WV": wv,
            "MASK2": mask2,
        })
    return in_maps


def unshard(results):
    """results: list of per-core {"OUT": [H, DA, L] bf16} -> [B, L, H*D] f32."""
    outs = np.stack(
        [np.asarray(r["OUT"], dtype=np.float32) for r in results]
    )                                                    # [B, H, DA, L]
    o = outs[:, :, :D, :] / outs[:, :, D:D + 1, :]       # [B, H, D, L]
    return np.ascontiguousarray(
        o.transpose(0, 3, 1, 2).reshape(B, L, H * D)
    ).astype(np.float32)


def run(inputs, **run_kwargs):
    """Compile + run on the 8 cores; returns (output, BassKernelResults)."""
    from concourse.bass_utils import run_bass_kernel_spmd

    nc = get_graph()
    in_maps = make_in_maps(
        inputs["Q_seq"], inputs["K_seq"], inputs["V_seq"],
        inputs["WQ"], inputs["WK"], inputs["WV"],
    )
    res = run_bass_kernel_spmd(
        nc, in_maps, core_ids=list(range(N_CORES)), **run_kwargs
    )
    return unshard(res.results), res


def kernel(Q_seq, K_seq, V_seq, WQ, WK, WV):
    out, _ = run({
        "Q_seq": Q_seq, "K_seq": K_seq, "V_seq": V_seq,
        "WQ": WQ, "WK": WK, "WV": WV,
    })
    return out


# revision 37
# speedup vs baseline: 1.0795x; 1.0795x over previous
"""Causal multi-head attention (B=8, L=1024, D_IN=512, H=8, D=64) on 8 TRN2
NeuronCores, data-parallel over batch (one batch element per core, no
collectives).

Every matmul runs in the SAME 64x128 row-tiled PE mode (no tiling-mode
switch drains), with the two row tiles T0 (SBUF partitions 0:64) and T8
(64:128) streaming CONCURRENTLY into different PSUM banks:

  proj:   qT/kT/v chunks contract K=512 as 4 K=64 subtiles per row tile;
          T0 accumulates bank A, T8 bank B, DVE adds A+B -> SBUF bf16.
  scores: head pair t lives on partition halves of qT/kT, so T0 computes
          head 2t and T8 head 2t+1 in parallel -> PSUM [128, 2, 512].
  exp:    ONE ScalarE activation over both heads' banks -> pexp SBUF bf16;
          causal diagonal 128x128 block masked by a DVE 0/1 multiply.
  AV:     cross passes: (T0: v_h keys-lo -> oT_h) || (T8: v_h' keys-hi ->
          oT_h'), then swapped, accumulating per-head oT [66, 512] banks
          over key tiles (ones columns in v carry the softmax denominator).

The attention loop is query-windowed (qc of 512 cols) and software-
pipelined: AV lags scores by TWO key tiles so the scalar-engine exp and
the DVE mask never block the PE's FIFO. Projection chunks for the next
head pair ride in the PE's idle slots (ScalarE is the saturated engine).

host: QsT/KsT/VsT = seq[b].T bf16; WQ pre-scaled by 1/sqrt(D);
      OUT[h, :64, :] / OUT[h, 64, :], transpose, concat heads.
"""

import numpy as np
import ml_dtypes

B, L, D_IN = 8, 1024, 512
H, D = 8, 64
DA = D + 2  # head dim + two ones columns (denominator; padded even so the
# bf16 lhsT slices stay 4-byte aligned -- odd column counts hang the HW)
N_CORES = 8
SCALE = 1.0 / np.sqrt(D).item()  # folded into WQ on the host
N_WARMUP = 22  # dummy matmuls to open the HAM clock gate during input DMA
# (sized so the warmup chain ends roughly when the first projection's
# inputs land ~18us in: keeps the PE continuously busy so HAM stays open)

_GRAPH_CACHE = {}


def build_attention_body(tc, qsT, ksT, vsT, wq, wk, wv, mask2, out):
    """Emit the per-core kernel into TileContext `tc` (APs per module doc)."""
    import contextlib

    import concourse.mybir as mybir

    nc = tc.nc
    fp32 = mybir.dt.float32
    bf16 = mybir.dt.bfloat16
    fp8 = mybir.dt.float8e4
    DR = mybir.MatmulPerfMode.DoubleRow
    EXP = mybir.ActivationFunctionType.Exp

    with contextlib.ExitStack() as ctx:
        const = ctx.enter_context(tc.tile_pool(name="const", bufs=1))
        sb = ctx.enter_context(tc.tile_pool(name="sb", bufs=1))
        ppool = ctx.enter_context(tc.tile_pool(name="ppool", bufs=1))
        stage = ctx.enter_context(tc.tile_pool(name="stage", bufs=1))
        psum = ctx.enter_context(tc.tile_pool(name="psum", bufs=2, space="PSUM"))

        # ---- ScalarE exp-table preload + PE warm-up racing the input DMAs --
        # (warmup borrows the "S" psum tag so proj chunks never wait on it)
        warm_sb = const.tile([128, 512], bf16)
        nc.vector.memset(warm_sb[:], 0.0)
        warm_out = const.tile([128, 8], bf16)
        nc.scalar.activation(warm_out[:], warm_sb[:, 0:8], EXP)
        pwarm = psum.tile([128, 2, 512], fp32, tag="S", bufs=2, name="pwarm")
        for i in range(N_WARMUP):
            nc.tensor.matmul(
                pwarm[:, i % 2, :], warm_sb[:, 0:128], warm_sb[:],
                start=True, stop=True, skip_group_check=True,
            )

        # ---- stage inputs into SBUF. Host pre-rearranged every tensor to
        # [128, ...] partition-major, so each DMA moves fully contiguous
        # 2-8 KB lines per partition. The two HWDGE rings (sync + ScalarE)
        # carry the q/v- and k-paths in parallel, seq tensors split in
        # halves ordered by first use so the first projection starts early.
        wq_r = wq.rearrange("p (kt n) -> p kt n", kt=4)
        wk_r = wk.rearrange("p (kt n) -> p kt n", kt=4)
        qsT_r = qsT.rearrange("p (kt h l) -> p kt h l", kt=4, h=2)
        ksT_r = ksT.rearrange("p (kt h l) -> p kt h l", kt=4, h=2)
        vsT_r = vsT.rearrange("p (kt h l) -> p kt h l", kt=4, h=2)
        wq_sb = const.tile([128, 4, 512], bf16)
        qsT_sb = const.tile([128, 4, L], bf16)
        wk_sb = const.tile([128, 4, 512], bf16)
        ksT_sb = const.tile([128, 4, L], bf16)
        wv_sb = const.tile([128, 4, 512], bf16)
        vsT_sb = const.tile([128, 4, L], bf16)
        mask_sb = const.tile([128, 2, 128], bf16)
        # first-window q/k slices lead each ring; v-path and second halves
        # follow (all DMAs compete for chip HBM bandwidth across the 8
        # cores, so ordering is best-effort)
        nc.sync.dma_start(wq_sb[:], wq_r[:, :, :])
        nc.scalar.dma_start(wk_sb[:], wk_r[:, :, :])
        nc.sync.dma_start(qsT_sb[:, :, 0:512], qsT_r[:, :, 0])
        nc.scalar.dma_start(ksT_sb[:, :, 0:512], ksT_r[:, :, 0])
        nc.sync.dma_start(wv_sb[:], wv.rearrange("p (kt n) -> p kt n", kt=4))
        nc.scalar.dma_start(ksT_sb[:, :, 512:L], ksT_r[:, :, 1])
        nc.sync.dma_start(vsT_sb[:, :, 0:512], vsT_r[:, :, 0])
        nc.scalar.dma_start(mask_sb[:], mask2[:, :, :])
        nc.sync.dma_start(qsT_sb[:, :, 512:L], qsT_r[:, :, 1])
        nc.sync.dma_start(vsT_sb[:, :, 512:L], vsT_r[:, :, 1])

        # ---- persistent activations -------------------------------------
        qT_sb = sb.tile([128, 4, L], bf16)   # [dout%128, pair, L]
        # kT zero-padded per head: kTz[:, z, t, :] holds head 2t+z's 64 dims
        # on its own partition half and ZEROS on the other, so score matmuls
        # contract K=128 -- the same PE tiling mode as every other matmul
        # (no 64x128 <-> 128x128 mode-switch drains on the PE)
        kTz_sb = sb.tile([128, 2, 4, L], bf16)
        v_sb = sb.tile([128, 8, H, DA], bf16)  # [j%128, j//128, head, d|1|1]
        # ones everywhere; proj overwrites [:, :, :, 0:64], cols 64:66 stay 1
        nc.vector.memset(v_sb[:], 1.0)
        # per-partition selectors: 1 on the head's own partition half, 0 on
        # the other -- the kT copy then zero-pads kTz with NO big memset
        # gating the first scores (0 * finite = 0 exactly)
        halfsel = const.tile([128, 2], fp32)
        nc.vector.memset(halfsel[:], 0.0)
        nc.vector.memset(halfsel[0:64, 0:1], 1.0)
        nc.vector.memset(halfsel[64:128, 1:2], 1.0)

        def proj_qk_chunk(t, which, nch):
            # one [128, 512] chunk of qT (which=0) / kT (which=1), pair t
            # (projections contract K=128 full-array; DVE can't add two PSUM
            # banks, so the 64x128 split would double the streamed columns)
            w_t, src = ((wq_sb, qsT_sb), (wk_sb, ksT_sb))[which]
            pq = psum.tile(
                [128, 512], fp32, tag="work", bufs=2,
                name=f"pq_{t}_{which}_{nch}",
            )
            cols = slice(nch * 512, (nch + 1) * 512)
            for kt in range(4):
                nc.tensor.matmul(
                    pq[:],
                    w_t[:, kt, t * 128:(t + 1) * 128],
                    src[:, kt, cols],
                    start=(kt == 0),
                    stop=(kt == 3),
                )
            if which == 0:
                nc.vector.tensor_copy(out=qT_sb[:, t, cols], in_=pq[:])
            else:
                # each head's 64 dims land in its slot, the other half
                # zeroed by the selector
                for z in range(2):
                    nc.vector.tensor_scalar_mul(
                        out=kTz_sb[:, z, t, cols], in0=pq[:],
                        scalar1=halfsel[:, z:z + 1],
                    )

        def proj_v(it):
            # v natural: v[i, n] = sum_k Vs[i, k] WV[k, n]; lhsT = VsT tile
            pv = psum.tile([128, 512], fp32, tag="work", bufs=2,
                           name=f"pv_{it}")
            for kt in range(4):
                nc.tensor.matmul(
                    pv[:],
                    vsT_sb[:, kt, it * 128:(it + 1) * 128],
                    wv_sb[:, kt, :],
                    start=(kt == 0),
                    stop=(kt == 3),
                )
            nc.vector.tensor_copy(
                out=v_sb[:, it, :, 0:D],
                in_=pv.rearrange("p (h d) -> p h d", h=H),
            )

        # proj work interleaved into the attention jt loops so the PE has
        # useful work while ScalarE (the critical engine) drains exps
        fillers = []
        for which in range(2):
            fillers.append(lambda w=which: proj_qk_chunk(0, w, 1))
        for it in range(4, 8):
            fillers.append(lambda it=it: proj_v(it))
        for t in range(1, 4):
            for which in range(2):
                for nch in range(2):
                    fillers.append(
                        (lambda t=t, w=which, n=nch: proj_qk_chunk(t, w, n))
                    )

        # spread proj fillers across the 48 attention steps (front-loading
        # them makes the first half PE-bound and the back half ScalarE-
        # starved); each lands before the pair that consumes it starts
        filler_steps = {0, 2, 3, 5, 6, 7, 8, 9, 10, 11,
                        14, 17, 20, 23, 26, 29, 32, 35}
        step_counter = [0]

        def emit_filler():
            if fillers and step_counter[0] in filler_steps:
                fillers.pop(0)()
            step_counter[0] += 1

        def emit_av(t, jt, last_jt, oT, pexp, qoff, cw):
            for hh in range(2):
                nc.tensor.matmul(
                    oT[hh][:, qoff:qoff + cw],
                    v_sb[:, jt, 2 * t + hh, :],
                    pexp[:, hh, 0:cw],
                    start=(jt == 0),
                    stop=(jt == last_jt),
                    skip_group_check=True,
                )

        def attention_pair(t):
            # pair 3 does the big window first so the kernel tail is short
            for qc in ((1, 0) if t == 3 else (0, 1)):
                oT = [
                    psum.tile([DA, 512], fp32, tag="oT", bufs=2,
                              name=f"oT_{t}_{qc}_{hh}")
                    for hh in range(2)
                ]
                last_jt = 4 * qc + 3
                pipe = []
                for jt in range(last_jt + 1):
                    j0 = 128 * jt
                    lo = max(j0, 512 * qc)
                    cw = 512 * qc + 512 - lo
                    qoff = lo - 512 * qc
                    diag = (lo == j0)
                    ps = psum.tile(
                        [128, 2, 512], fp32, tag="S", bufs=2,
                        name=f"S_{t}_{qc}_{jt}",
                    )
                    for hh in range(2):
                        nc.tensor.matmul(
                            ps[:, hh, 0:cw],
                            kTz_sb[:, hh, t, j0:j0 + 128],
                            qT_sb[:, t, lo:lo + cw],
                            start=True,
                            stop=True,
                            skip_group_check=True,
                        )
                    pexp = ppool.tile(
                        [128, 2, 512], bf16, tag="P", bufs=6,
                        name=f"P_{t}_{qc}_{jt}",
                    )
                    nc.scalar.activation(pexp[:, :, 0:cw], ps[:, :, 0:cw], EXP)
                    if diag:
                        # causal 0/1 mask on the diagonal 128x128 block
                        nc.vector.tensor_mul(
                            pexp[:, :, 0:128], pexp[:, :, 0:128], mask_sb[:]
                        )
                    pipe.append((t, jt, last_jt, oT, pexp, qoff, cw))
                    emit_filler()
                    if len(pipe) > 2:  # AV lags scores by two key tiles
                        emit_av(*pipe.pop(0))
                while pipe:
                    emit_av(*pipe.pop(0))
                for hh in range(2):
                    o_st = stage.tile(
                        [DA, 512], bf16, tag="ost", bufs=4,
                        name=f"ost_{t}_{qc}_{hh}",
                    )
                    nc.vector.tensor_copy(out=o_st[:], in_=oT[hh][:])
                    nc.sync.dma_start(
                        out[2 * t + hh, :, 512 * qc:512 * qc + 512], o_st[:]
                    )

        # emit: pair-0 q/k first-window projections (scores start earliest),
        # then v tiles 0..3 (first AVs); the rest ride the filler queue
        for which in range(2):
            proj_qk_chunk(0, which, 0)
        for it in range(4):
            proj_v(it)
        for t in range(4):
            attention_pair(t)


def _build_graph():
    import concourse.mybir as mybir
    import concourse.tile as tile
    from concourse import bacc

    nc = bacc.Bacc("TRN2", target_bir_lowering=False)
    bf16 = mybir.dt.bfloat16
    # all inputs pre-rearranged on the host to partition-major [128, ...]
    qsT = nc.dram_tensor("QsT", (128, 4 * L), bf16, kind="ExternalInput")
    ksT = nc.dram_tensor("KsT", (128, 4 * L), bf16, kind="ExternalInput")
    vsT = nc.dram_tensor("VsT", (128, 4 * L), bf16, kind="ExternalInput")
    wq = nc.dram_tensor("WQ", (128, 4 * 512), bf16, kind="ExternalInput")
    wk = nc.dram_tensor("WK", (128, 4 * 512), bf16, kind="ExternalInput")
    wv = nc.dram_tensor("WV", (128, 4 * 512), bf16, kind="ExternalInput")
    mask2 = nc.dram_tensor("MASK2", (128, 2, 128), bf16, kind="ExternalInput")
    out = nc.dram_tensor("OUT", (H, DA, L), bf16, kind="ExternalOutput")

    with tile.TileContext(nc) as tc:
        build_attention_body(
            tc, qsT[:], ksT[:], vsT[:], wq[:], wk[:], wv[:], mask2[:], out[:],
        )
    nc.compile()
    return nc


def get_graph():
    if "nc" not in _GRAPH_CACHE:
        _GRAPH_CACHE["nc"] = _build_graph()
    return _GRAPH_CACHE["nc"]


def _prep_seq(x):
    """[L, 512] -> partition-major [128, 4*L] bf16 (x.T tiled by 128 rows)."""
    xT = np.asarray(x, np.float32).T.reshape(4, 128, L)
    return np.ascontiguousarray(
        xT.transpose(1, 0, 2).reshape(128, 4 * L)
    ).astype(ml_dtypes.bfloat16)


def _prep_w(w, scale=1.0):
    """[512, 512] -> partition-major [128, 4*512] bf16."""
    w32 = (np.asarray(w, np.float32) * scale).reshape(4, 128, 512)
    return np.ascontiguousarray(
        w32.transpose(1, 0, 2).reshape(128, 4 * 512)
    ).astype(ml_dtypes.bfloat16)


def make_in_maps(Q_seq, K_seq, V_seq, WQ, WK, WV):
    bf = ml_dtypes.bfloat16
    # fold the softmax 1/sqrt(D) into WQ so no scale is needed on-device
    # (fp8 projections were tried and give 4.5e-2 rel err -- over budget)
    wq = _prep_w(WQ, SCALE)
    wk = _prep_w(WK)
    wv = _prep_w(WV)
    # keep-mask in S^T block coords, duplicated per head of the pair:
    # keep key <= query  <=>  row r (key) <= col c (query)
    m = np.triu(np.ones((128, 128), np.float32))
    mask2 = np.ascontiguousarray(
        np.broadcast_to(m[:, None, :], (128, 2, 128))
    ).astype(bf)
    in_maps = []
    for b in range(N_CORES):
        in_maps.append({
            "QsT": _prep_seq(Q_seq[b]),
            "KsT": _prep_seq(K_seq[b]),
            "VsT": _prep_seq(V_seq[b]),
            "WQ": wq,
            "WK": wk,
            "WV": wv,
            "MASK2": mask2,
        })
    return in_maps


def unshard(results):
    """results: list of per-core {"OUT": [H, DA, L] bf16} -> [B, L, H*D] f32."""
    outs = np.stack(
        [np.asarray(r["OUT"], dtype=np.float32) for r in results]
    )                                                    # [B, H, DA, L]
    o = outs[:, :, :D, :] / outs[:, :, D:D + 1, :]       # [B, H, D, L]
    return np.ascontiguousarray(
        o.transpose(0, 3, 1, 2).reshape(B, L, H * D)
    ).astype(np.float32)


def run(inputs, **run_kwargs):
    """Compile + run on the 8 cores; returns (output, BassKernelResults)."""
    from concourse.bass_utils import run_bass_kernel_spmd

    nc = get_graph()
    in_maps = make_in_maps(
        inputs["Q_seq"], inputs["K_seq"], inputs["V_seq"],
        inputs["WQ"], inputs["WK"], inputs["WV"],
    )
    res = run_bass_kernel_spmd(
        nc, in_maps, core_ids=list(range(N_CORES)), **run_kwargs
    )
    return unshard(res.results), res


def kernel(Q_seq, K_seq, V_seq, WQ, WK, WV):
    out, _ = run({
        "Q_seq": Q_seq, "K_seq": K_seq, "V_seq": V_seq,
        "WQ": WQ, "WK": WK, "WV": WV,
    })
    return out
